# Initial kernel scaffold
#
"""SupCon cluster-memory loss kernel for 8 TRN2 NeuronCores — raw bass.

Math (per core, N-shard of 1024 bank rows x 4 (anchor, bank) combos):
  sumexp[i] = sum_j exp((x_a . mem_b_j)/T - shift_b)
via fp8 DoubleRow matmuls + ScalarE Exp + VectorE row-sum, as 8
sequential supergroups (b, mt, a) of 6 matmuls each so each
supergroup's Exp+reduce overlaps the following supergroups' matmuls.
The positives term (pos_i = x_i . G_i over <=1 matching bank row) is
pure index bookkeeping + a [256,768] dot on the HOST — no device work.

Hand-scheduled without TileContext:
- Per-DMA completion semaphores (+16 on completion); waits deduped
  along each engine's program order (sem values are monotone).
- DMA: whole [128, 6kt, 512] column blocks (393KB, 3KB/partition runs
  -> near-peak queue bandwidth), x split across both HWDGE rings.
- Garbage warmup matmuls keep the PE continuously busy from t~0.7us so
  the HAM clock ramp completes during the DMA fill.
- The last supergroup's Exp is split into nt halves with fused row
  accumulation; the first half overlaps the supergroup's remaining
  matmuls (different PSUM banks), so the post-stream tail is ~1.5us.
- Epilogue: just two ring drains.  The neuronxcc custom-kernel lowering
  appends its own all-engine barrier plus a full sweep zeroing sems
  3..255 (ours included) on every execution — a fixed ~8us toolchain
  tax that also makes an explicit barrier + sem clear redundant.
"""

import ml_dtypes
import numpy as np

import concourse.bacc as bacc
import concourse.mybir as mybir
from concourse.bass_utils import run_bass_kernel_spmd

BF16_NP = ml_dtypes.bfloat16

B = 256          # anchor batch per modality
N = 8192         # memory bank rows
D = 768          # feature dim
NCORES = 8
NS = N // NCORES     # 1024 bank rows per core
KT = D // 128        # 6 contraction tiles
MT = B // 128        # 2 anchor partition tiles
SUPCON_T = 0.07

F32 = mybir.dt.float32
FP8 = mybir.dt.float8e4
FP8_NP = ml_dtypes.float8_e4m3
FP8_SCALE = 16.0

# Supergroup schedule: (b, mt, a).  Bank-1 blocks arrive first on the
# Scalar ring, bank-0 on the Sync ring.
SG_ORDER = [(1, 0, 0), (1, 0, 1), (1, 1, 0), (1, 1, 1),
            (0, 0, 0), (0, 0, 1), (0, 1, 0), (0, 1, 1)]
NWU = 28         # warmup matmuls ([128, 256] free dim)
NOUT = 9         # rs cols: sg0..sg6 -> 0..6, sg7 halves -> 7,8

_NC_CACHE = {}


def _build_nc():
    nc = bacc.Bacc("TRN2", target_bir_lowering=False, debug=False,
                   num_devices=NCORES)

    # xT split by anchor half a: [2][128, KT, B] fp8.
    xT = nc.dram_tensor("xT", [2, 128, KT, B], FP8, kind="ExternalInput").ap()
    # memB[b][nt] = [128, KT, 512] fp8 column block of bank b.
    memB = nc.dram_tensor("memB", [2, 2, 128, KT, 512], FP8,
                          kind="ExternalInput").ap()
    nshift_h = nc.dram_tensor("nshift", [128, MT, 2, 2], F32,
                              kind="ExternalInput").ap()
    res = nc.dram_tensor("res", [128, NOUT], F32, kind="ExternalOutput").ap()

    x_sb = [nc.alloc_sbuf_tensor(f"x{a}", [128, KT, B], FP8).ap()
            for a in range(2)]
    blk = {(b, nt): nc.alloc_sbuf_tensor(f"m{b}{nt}", [128, KT, 512], FP8).ap()
           for b in range(2) for nt in range(2)}
    shift_t = nc.alloc_sbuf_tensor("shift", [128, MT, 2, 2], F32).ap()
    rs = nc.alloc_sbuf_tensor("rs", [128, NOUT], F32).ap()
    wu_w = nc.alloc_sbuf_tensor("wu_w", [128, 128], FP8).ap()
    wu_r = nc.alloc_sbuf_tensor("wu_r", [128, 256], FP8).ap()
    acc = [nc.alloc_psum_tensor(f"acc{i}", [128, 1024], F32).ap()
           for i in range(4)]

    sems = []

    def sem(name):
        s = nc.alloc_semaphore(name)
        sems.append(s)
        return s

    c10 = [sem(f"c10_{i}") for i in range(4)]   # scalar ring: b1n0, x1, shift, b0n0
    c1 = [sem(f"c1_{i}") for i in range(3)]     # sync ring: x0, b1n1, b0n1
    s_mm = sem("s_mm")      # supergroup matmul accumulation done (Tensor)
    s_exp = sem("s_exp")    # supergroup Exp done (Scalar)
    s_red = sem("s_red")    # supergroup reduce done (Vector)
    s_acc = sem("s_acc")    # sg7 accum Exps + reads retired (Scalar)
    s_mm7 = sem("s_mm7")    # sg7 first-half (nt0) accumulation done (Tensor)
    s_fin = sem("s_fin")    # output DMA done

    # ---- DMA: whole-block transfers (393KB, best ring throughput), each
    # queue in consumption order of the nt-sequential supergroup schedule.
    # The matmul stream waits once, up front, for b1n0+x0 — a single
    # pre-stream gap keeps the HAM duty ramp intact, unlike repeated
    # mid-stream chunk stalls.
    nc.scalar.dma_start(out=blk[1, 0], in_=memB[1, 0]).then_inc(c10[0], 16)
    nc.scalar.dma_start(out=x_sb[1], in_=xT[1]).then_inc(c10[1], 16)
    nc.scalar.dma_start(out=shift_t, in_=nshift_h).then_inc(c10[2], 16)
    nc.scalar.dma_start(out=blk[0, 0], in_=memB[0, 0]).then_inc(c10[3], 16)

    nc.sync.dma_start(out=x_sb[0], in_=xT[0]).then_inc(c1[0], 16)
    nc.sync.dma_start(out=blk[1, 1], in_=memB[1, 1]).then_inc(c1[1], 16)
    nc.sync.dma_start(out=blk[0, 1], in_=memB[0, 1]).then_inc(c1[2], 16)

    # ---- Vector: supergroup reduces.
    for si in range(7):
        nc.vector.wait_ge(s_exp, si + 1)
        nc.vector.tensor_reduce(out=rs[:, si:si + 1], in_=acc[si % 4],
                                axis=mybir.AxisListType.X,
                                op=mybir.AluOpType.add).then_inc(s_red, 1)

    # ---- Tensor: warmups, then 8 sequential supergroups.  The warmup
    # operands are uninitialized SBUF garbage on purpose — results are
    # discarded (sg0's start=True resets the PSUM region) and arbitrary
    # fp8 bits can't fault the PE — so the chain starts right after the
    # init barrier with no memset dependency, maximizing HAM ramp time.
    for _ in range(NWU):
        nc.tensor.matmul(acc[0][:, 0:256], wu_w, wu_r, start=True, stop=True)

    hi = {}

    def twait(s, v):
        if hi.get(s.num, 0) < v:
            hi[s.num] = v
            nc.tensor.wait_ge(s, v)

    BLK_SEM = {(1, 0): c10[0], (1, 1): c1[1], (0, 0): c10[3], (0, 1): c1[2]}
    for si, (b, mt, a) in enumerate(SG_ORDER):
        for nt in range(2):
            for kp in range(KT // 2):
                if nt == 0 and kp == 0 and si >= 4:
                    twait(s_red, si - 3)          # psum tile WAR
                twait(BLK_SEM[b, nt], 16)
                twait(c10[1] if a == 1 else c1[0], 16)   # x half
                mm = nc.tensor.matmul(
                    acc[si % 4][:, nt * 512:(nt + 1) * 512],
                    x_sb[a][:, 2 * kp:2 * kp + 2, mt * 128:(mt + 1) * 128],
                    blk[b, nt][:, 2 * kp:2 * kp + 2],
                    start=(kp == 0), stop=(kp == KT // 2 - 1),
                    perf_mode=mybir.MatmulPerfMode.DoubleRow)
            if si == 7 and nt == 0:
                mm.then_inc(s_mm7, 1)
        mm.then_inc(s_mm, 1)

    # ---- Scalar: supergroup Exps (after its DMA posts above).
    scale = 1.0 / (SUPCON_T * FP8_SCALE * FP8_SCALE)
    nc.scalar.wait_ge(c10[2], 16)                 # shift loaded
    for si, (b, mt, a) in enumerate(SG_ORDER):
        bias = shift_t[:, mt, a, b:b + 1]
        if si < 7:
            nc.scalar.wait_ge(s_mm, si + 1)
            nc.scalar.activation(
                out=acc[si % 4], in_=acc[si % 4],
                func=mybir.ActivationFunctionType.Exp,
                bias=bias, scale=scale).then_inc(s_exp, 1)
        else:
            # Last supergroup: two half Exps with fused row-accumulation
            # straight into rs.  The nt0 half is gated on s_mm7, so it
            # overlaps sg7's remaining nt1 matmuls (different PSUM banks).
            # then_inc on the activation lands on its final lowered
            # sub-instruction (the ACTIVATION_READ_ACCUMULATOR that writes
            # rs), so the output DMA's s_acc wait covers the rs writes.
            nc.scalar.wait_ge(s_mm7, 1)
            nc.scalar.activation(
                out=acc[3][:, 0:512], in_=acc[3][:, 0:512],
                func=mybir.ActivationFunctionType.Exp,
                bias=bias, scale=scale,
                accum_out=rs[:, 7:8]).then_inc(s_acc, 1)
            nc.scalar.wait_ge(s_mm, 8)
            nc.scalar.activation(
                out=acc[3][:, 512:1024], in_=acc[3][:, 512:1024],
                func=mybir.ActivationFunctionType.Exp,
                bias=bias, scale=scale,
                accum_out=rs[:, 8:9]).then_inc(s_acc, 1)

    # ---- Sync: output DMA once every rs column is written (7 supergroup
    # reduces on Vector, sg7's accumulated halves on Scalar).
    nc.sync.wait_ge(s_red, 7)
    nc.sync.wait_ge(s_acc, 2)
    nc.sync.dma_start(out=res, in_=rs).then_inc(s_fin, 16)

    # ---- Epilogue: drain the two HWDGE rings.  No explicit barrier or
    # semaphore clear: the neuronxcc custom-kernel lowering appends its own
    # all-engine barrier plus a full sweep that zeroes sems 3..255 (ours
    # included) on every execution, so re-runs start clean either way.
    nc.sync.drain()
    nc.scalar.drain()

    nc.compile()
    return nc


def get_nc():
    if "nc" not in _NC_CACHE:
        _NC_CACHE["nc"] = _build_nc()
    return _NC_CACHE["nc"]


def _l2norm(x):
    n = np.linalg.norm(x, axis=-1, keepdims=True)
    return x / np.maximum(n, 1e-12)


def _gather_positives(feats_b, lab_a, mlab_b):
    """G[i] = sum of bank rows whose prototype label == lab_a[i]."""
    G = np.zeros((B, D), np.float32)
    if np.unique(mlab_b).size == mlab_b.size:
        inv = np.full(1 << 14, -1, np.int64)
        inv[mlab_b] = np.arange(mlab_b.size)
        idx = inv[np.clip(lab_a, 0, (1 << 14) - 1)]
        valid = idx >= 0
        G[valid] = feats_b[idx[valid]]
    else:
        by_label = np.zeros((1 << 14, D), np.float32)
        np.add.at(by_label, mlab_b, feats_b)
        G[:] = by_label[np.clip(lab_a, 0, (1 << 14) - 1)]
    return G


def make_in_maps(inputs_rgb, inputs_ir, targets_rgb, targets_ir,
                 features_rgb, features_ir,
                 prototype_labels_rgb, prototype_labels_ir):
    x = [_l2norm(np.asarray(inputs_rgb, np.float32)),
         _l2norm(np.asarray(inputs_ir, np.float32))]
    feats = [np.asarray(features_rgb, np.float32),
             np.asarray(features_ir, np.float32)]
    lab = [np.asarray(targets_rgb).astype(np.int64),
           np.asarray(targets_ir).astype(np.int64)]
    mlab = [np.asarray(prototype_labels_rgb).astype(np.int64),
            np.asarray(prototype_labels_ir).astype(np.int64)]

    # xT[a] = [128, KT, B]: x[a].T tiled over kt.
    xT = np.empty([2, 128, KT, B], np.float32)
    for a in range(2):
        xT[a] = (x[a].T.reshape(KT, 128, B) * FP8_SCALE).transpose(1, 0, 2)
    xT = np.ascontiguousarray(xT).astype(FP8_NP)

    bank_max = [float(np.sqrt((feats[b] ** 2).sum(axis=1).max()))
                for b in range(2)]
    shift = np.empty((B, 2, 2), np.float64)                   # [i, a, b]
    if max(bank_max) <= 2.0:
        for b in range(2):
            shift[:, :, b] = bank_max[b] / SUPCON_T
    else:
        for a in range(2):
            for b in range(2):
                shift[:, a, b] = (x[a] @ feats[b].T).max(axis=1) / SUPCON_T
    nshift = np.ascontiguousarray(
        (-shift).reshape(MT, 128, 2, 2).transpose(1, 0, 2, 3)).astype(np.float32)

    # Host-side positives: pos[a][b][i] = x[a][i] . G_ab[i].
    pos = np.empty((2, 2, B), np.float64)
    for a in range(2):
        for b in range(2):
            G = _gather_positives(feats[b], lab[a], mlab[b])
            pos[a, b] = (x[a].astype(np.float64) *
                         G.astype(np.float64)).sum(axis=1)

    in_maps = []
    for c in range(NCORES):
        memB = np.empty([2, 2, 128, KT, 512], FP8_NP)
        for b in range(2):
            for nt in range(2):
                b_rows = feats[b][c * NS + nt * 512:c * NS + (nt + 1) * 512, :]
                memB[b, nt] = (b_rows.T * FP8_SCALE).reshape(
                    KT, 128, 512).transpose(1, 0, 2).astype(FP8_NP)
        in_maps.append({
            "xT": xT,
            "memB": memB,
            "nshift": nshift,
        })
    return in_maps, (shift, pos)


def combine(results, aux, targets_rgb, targets_ir,
            prototype_labels_rgb, prototype_labels_ir):
    shift, pos = aux
    rs = np.stack([np.asarray(r["res"], np.float64) for r in results])
    rs_sum = rs.sum(axis=0)                                    # [128, NOUT]
    sumexp = np.zeros((B, 4), np.float64)
    for si, (b, mt, a) in enumerate(SG_ORDER):
        c = a * 2 + b
        col = rs_sum[:, si] if si < 7 else rs_sum[:, 7] + rs_sum[:, 8]
        sumexp[mt * 128:(mt + 1) * 128, c] = col

    lab = [np.asarray(targets_rgb).astype(np.int64),
           np.asarray(targets_ir).astype(np.int64)]
    mlab = [np.asarray(prototype_labels_rgb).astype(np.int64),
            np.asarray(prototype_labels_ir).astype(np.int64)]

    losses = np.zeros(4, np.float64)
    for a in range(2):
        for b in range(2):
            c = a * 2 + b
            lse = shift[:, a, b] + np.log(sumexp[:, c])
            cnt = np.bincount(mlab[b], minlength=1 << 14)[
                np.clip(lab[a], 0, (1 << 14) - 1)].astype(np.float64)
            mlpp = (pos[a, b] / SUPCON_T - cnt * lse) / np.maximum(cnt, 1.0)
            losses[c] = -mlpp.mean()

    loss_contr = losses[0] + losses[3]        # (rgb,rgb) + (ir,ir)
    loss_cross = losses[1] + losses[2]        # (rgb,ir)  + (ir,rgb)
    return np.asarray([loss_contr, loss_cross], np.float32)


def run_device(in_maps, **kwargs):
    return run_bass_kernel_spmd(get_nc(), in_maps,
                                core_ids=list(range(NCORES)), **kwargs)


def kernel(inputs_rgb, inputs_ir, targets_rgb, targets_ir,
           features_rgb, features_ir,
           prototype_labels_rgb, prototype_labels_ir):
    in_maps, aux = make_in_maps(inputs_rgb, inputs_ir, targets_rgb,
                                targets_ir, features_rgb, features_ir,
                                prototype_labels_rgb, prototype_labels_ir)
    results = run_device(in_maps).results
    return combine(results, aux, targets_rgb, targets_ir,
                   prototype_labels_rgb, prototype_labels_ir)



# revision 1
# speedup vs baseline: 1.0350x; 1.0350x over previous
"""SupCon cluster-memory loss kernel for 8 TRN2 NeuronCores — raw bass.

Math (per core, N-shard of 1024 bank rows x 4 (anchor, bank) combos):
  sumexp[i] = sum_j exp((x_a . mem_b_j)/T - shift_b)
via fp8 DoubleRow matmuls + ScalarE Exp + VectorE row-sum, as 8
sequential supergroups (b, mt, a) of 6 matmuls each so each
supergroup's Exp+reduce overlaps the following supergroups' matmuls.
The positives term (pos_i = x_i . G_i over <=1 matching bank row) is
pure index bookkeeping + a [256,768] dot on the HOST — no device work.

Hand-scheduled without TileContext:
- Per-DMA completion semaphores (+16 on completion); waits deduped
  along each engine's program order (sem values are monotone).
- DMA: whole [128, 6kt, 512] column blocks (393KB, 3KB/partition runs
  -> near-peak queue bandwidth), x split across both HWDGE rings.
- Garbage warmup matmuls keep the PE continuously busy from t~0.7us so
  the HAM clock ramp completes during the DMA fill.
- The last supergroup's Exp is split into nt halves with fused row
  accumulation; the first half overlaps the supergroup's remaining
  matmuls (different PSUM banks), so the post-stream tail is ~1.5us.
- Epilogue: just two ring drains.  The neuronxcc custom-kernel lowering
  appends its own all-engine barrier plus a full sweep zeroing sems
  3..255 (ours included) on every execution — a fixed ~8us toolchain
  tax that also makes an explicit barrier + sem clear redundant.
"""

import ml_dtypes
import numpy as np

import concourse.bacc as bacc
import concourse.mybir as mybir
from concourse.bass_utils import run_bass_kernel_spmd

BF16_NP = ml_dtypes.bfloat16

B = 256          # anchor batch per modality
N = 8192         # memory bank rows
D = 768          # feature dim
NCORES = 8
NS = N // NCORES     # 1024 bank rows per core
KT = D // 128        # 6 contraction tiles
MT = B // 128        # 2 anchor partition tiles
SUPCON_T = 0.07

F32 = mybir.dt.float32
FP8 = mybir.dt.float8e4
FP8_NP = ml_dtypes.float8_e4m3
FP8_SCALE = 16.0

# Supergroup schedule: (b, mt, a).  Bank-1 blocks arrive first on the
# Scalar ring, bank-0 on the Sync ring.
SG_ORDER = [(1, 0, 0), (1, 0, 1), (1, 1, 0), (1, 1, 1),
            (0, 0, 0), (0, 0, 1), (0, 1, 0), (0, 1, 1)]
NWU = 28         # warmup matmuls ([128, 256] free dim)
NOUT = 9         # rs cols: sg0..sg6 -> 0..6, sg7 halves -> 7,8

_NC_CACHE = {}


def _build_nc():
    nc = bacc.Bacc("TRN2", target_bir_lowering=False, debug=False,
                   num_devices=NCORES)

    # xT split by anchor half a: [2][128, KT, B] fp8.
    xT = nc.dram_tensor("xT", [2, 128, KT, B], FP8, kind="ExternalInput").ap()
    # memB[b][nt] = [128, KT, 512] fp8 column block of bank b.
    memB = nc.dram_tensor("memB", [2, 2, 128, KT, 512], FP8,
                          kind="ExternalInput").ap()
    nshift_h = nc.dram_tensor("nshift", [128, MT, 2, 2], F32,
                              kind="ExternalInput").ap()
    res = nc.dram_tensor("res", [128, NOUT], F32, kind="ExternalOutput").ap()

    x_sb = [nc.alloc_sbuf_tensor(f"x{a}", [128, KT, B], FP8).ap()
            for a in range(2)]
    blk = {(b, nt): nc.alloc_sbuf_tensor(f"m{b}{nt}", [128, KT, 512], FP8).ap()
           for b in range(2) for nt in range(2)}
    shift_t = nc.alloc_sbuf_tensor("shift", [128, MT, 2, 2], F32).ap()
    rs = nc.alloc_sbuf_tensor("rs", [128, NOUT], F32).ap()
    wu_w = nc.alloc_sbuf_tensor("wu_w", [128, 128], FP8).ap()
    wu_r = nc.alloc_sbuf_tensor("wu_r", [128, 256], FP8).ap()
    acc = [nc.alloc_psum_tensor(f"acc{i}", [128, 1024], F32).ap()
           for i in range(4)]

    sems = []

    def sem(name):
        s = nc.alloc_semaphore(name)
        sems.append(s)
        return s

    c10 = [sem(f"c10_{i}") for i in range(4)]   # scalar ring: b1n0, x1, shift, b0n0
    c1 = [sem(f"c1_{i}") for i in range(3)]     # sync ring: x0, b1n1, b0n1
    s_mm = sem("s_mm")      # supergroup matmul accumulation done (Tensor)
    s_exp = sem("s_exp")    # supergroup Exp done (Scalar)
    s_red = sem("s_red")    # supergroup reduce done (Vector)
    s_acc = sem("s_acc")    # sg7 accum Exps + reads retired (Scalar)
    s_mm7 = sem("s_mm7")    # sg7 first-half (nt0) accumulation done (Tensor)
    s_fin = sem("s_fin")    # output DMA done

    # ---- DMA: whole-block transfers (393KB, best ring throughput), each
    # queue in consumption order of the nt-sequential supergroup schedule.
    # The matmul stream waits once, up front, for b1n0+x0 — a single
    # pre-stream gap keeps the HAM duty ramp intact, unlike repeated
    # mid-stream chunk stalls.
    nc.scalar.dma_start(out=blk[1, 0], in_=memB[1, 0]).then_inc(c10[0], 16)
    nc.scalar.dma_start(out=x_sb[1], in_=xT[1]).then_inc(c10[1], 16)
    nc.scalar.dma_start(out=shift_t, in_=nshift_h).then_inc(c10[2], 16)
    nc.scalar.dma_start(out=blk[0, 0], in_=memB[0, 0]).then_inc(c10[3], 16)

    nc.sync.dma_start(out=x_sb[0], in_=xT[0]).then_inc(c1[0], 16)
    nc.sync.dma_start(out=blk[1, 1], in_=memB[1, 1]).then_inc(c1[1], 16)
    nc.sync.dma_start(out=blk[0, 1], in_=memB[0, 1]).then_inc(c1[2], 16)

    # ---- Vector: supergroup reduces.
    for si in range(7):
        nc.vector.wait_ge(s_exp, si + 1)
        nc.vector.tensor_reduce(out=rs[:, si:si + 1], in_=acc[si % 4],
                                axis=mybir.AxisListType.X,
                                op=mybir.AluOpType.add).then_inc(s_red, 1)

    # ---- Tensor: warmups, then 8 sequential supergroups.  The warmup
    # operands are uninitialized SBUF garbage on purpose — results are
    # discarded (sg0's start=True resets the PSUM region) and arbitrary
    # fp8 bits can't fault the PE — so the chain starts right after the
    # init barrier with no memset dependency, maximizing HAM ramp time.
    for _ in range(NWU):
        nc.tensor.matmul(acc[0][:, 0:256], wu_w, wu_r, start=True, stop=True)

    hi = {}

    def twait(s, v):
        if hi.get(s.num, 0) < v:
            hi[s.num] = v
            nc.tensor.wait_ge(s, v)

    BLK_SEM = {(1, 0): c10[0], (1, 1): c1[1], (0, 0): c10[3], (0, 1): c1[2]}
    for si, (b, mt, a) in enumerate(SG_ORDER):
        for nt in range(2):
            for kp in range(KT // 2):
                if nt == 0 and kp == 0 and si >= 4:
                    twait(s_red, si - 3)          # psum tile WAR
                twait(BLK_SEM[b, nt], 16)
                twait(c10[1] if a == 1 else c1[0], 16)   # x half
                mm = nc.tensor.matmul(
                    acc[si % 4][:, nt * 512:(nt + 1) * 512],
                    x_sb[a][:, 2 * kp:2 * kp + 2, mt * 128:(mt + 1) * 128],
                    blk[b, nt][:, 2 * kp:2 * kp + 2],
                    start=(kp == 0), stop=(kp == KT // 2 - 1),
                    perf_mode=mybir.MatmulPerfMode.DoubleRow)
            if si == 7 and nt == 0:
                mm.then_inc(s_mm7, 1)
        mm.then_inc(s_mm, 1)

    # ---- Scalar: supergroup Exps (after its DMA posts above).
    scale = 1.0 / (SUPCON_T * FP8_SCALE * FP8_SCALE)
    nc.scalar.wait_ge(c10[2], 16)                 # shift loaded
    for si, (b, mt, a) in enumerate(SG_ORDER):
        bias = shift_t[:, mt, a, b:b + 1]
        if si < 7:
            nc.scalar.wait_ge(s_mm, si + 1)
            nc.scalar.activation(
                out=acc[si % 4], in_=acc[si % 4],
                func=mybir.ActivationFunctionType.Exp,
                bias=bias, scale=scale).then_inc(s_exp, 1)
        else:
            # Last supergroup: two half Exps with fused row-accumulation
            # straight into rs.  The nt0 half is gated on s_mm7, so it
            # overlaps sg7's remaining nt1 matmuls (different PSUM banks).
            # then_inc on the activation lands on its final lowered
            # sub-instruction (the ACTIVATION_READ_ACCUMULATOR that writes
            # rs), so the output DMA's s_acc wait covers the rs writes.
            nc.scalar.wait_ge(s_mm7, 1)
            nc.scalar.activation(
                out=acc[3][:, 0:512], in_=acc[3][:, 0:512],
                func=mybir.ActivationFunctionType.Exp,
                bias=bias, scale=scale,
                accum_out=rs[:, 7:8]).then_inc(s_acc, 1)
            nc.scalar.wait_ge(s_mm, 8)
            nc.scalar.activation(
                out=acc[3][:, 512:1024], in_=acc[3][:, 512:1024],
                func=mybir.ActivationFunctionType.Exp,
                bias=bias, scale=scale,
                accum_out=rs[:, 8:9]).then_inc(s_acc, 1)

    # ---- Sync: output DMA once every rs column is written (7 supergroup
    # reduces on Vector, sg7's accumulated halves on Scalar).
    nc.sync.wait_ge(s_red, 7)
    nc.sync.wait_ge(s_acc, 2)
    nc.sync.dma_start(out=res, in_=rs).then_inc(s_fin, 16)

    # ---- Epilogue: drain the two HWDGE rings.  No explicit barrier or
    # semaphore clear: the neuronxcc custom-kernel lowering appends its own
    # all-engine barrier plus a full sweep that zeroes sems 3..255 (ours
    # included) on every execution, so re-runs start clean either way.
    nc.sync.drain()
    nc.scalar.drain()

    nc.compile()
    return nc


def get_nc():
    if "nc" not in _NC_CACHE:
        _NC_CACHE["nc"] = _build_nc()
    return _NC_CACHE["nc"]


def _l2norm(x):
    n = np.linalg.norm(x, axis=-1, keepdims=True)
    return x / np.maximum(n, 1e-12)


def _gather_positives(feats_b, lab_a, mlab_b):
    """G[i] = sum of bank rows whose prototype label == lab_a[i]."""
    G = np.zeros((B, D), np.float32)
    if np.unique(mlab_b).size == mlab_b.size:
        inv = np.full(1 << 14, -1, np.int64)
        inv[mlab_b] = np.arange(mlab_b.size)
        idx = inv[np.clip(lab_a, 0, (1 << 14) - 1)]
        valid = idx >= 0
        G[valid] = feats_b[idx[valid]]
    else:
        by_label = np.zeros((1 << 14, D), np.float32)
        np.add.at(by_label, mlab_b, feats_b)
        G[:] = by_label[np.clip(lab_a, 0, (1 << 14) - 1)]
    return G


def make_in_maps(inputs_rgb, inputs_ir, targets_rgb, targets_ir,
                 features_rgb, features_ir,
                 prototype_labels_rgb, prototype_labels_ir):
    x = [_l2norm(np.asarray(inputs_rgb, np.float32)),
         _l2norm(np.asarray(inputs_ir, np.float32))]
    feats = [np.asarray(features_rgb, np.float32),
             np.asarray(features_ir, np.float32)]
    lab = [np.asarray(targets_rgb).astype(np.int64),
           np.asarray(targets_ir).astype(np.int64)]
    mlab = [np.asarray(prototype_labels_rgb).astype(np.int64),
            np.asarray(prototype_labels_ir).astype(np.int64)]

    # xT[a] = [128, KT, B]: x[a].T tiled over kt.
    xT = np.empty([2, 128, KT, B], np.float32)
    for a in range(2):
        xT[a] = (x[a].T.reshape(KT, 128, B) * FP8_SCALE).transpose(1, 0, 2)
    xT = np.ascontiguousarray(xT).astype(FP8_NP)

    bank_max = [float(np.sqrt((feats[b] ** 2).sum(axis=1).max()))
                for b in range(2)]
    shift = np.empty((B, 2, 2), np.float64)                   # [i, a, b]
    if max(bank_max) <= 2.0:
        for b in range(2):
            shift[:, :, b] = bank_max[b] / SUPCON_T
    else:
        for a in range(2):
            for b in range(2):
                shift[:, a, b] = (x[a] @ feats[b].T).max(axis=1) / SUPCON_T
    nshift = np.ascontiguousarray(
        (-shift).reshape(MT, 128, 2, 2).transpose(1, 0, 2, 3)).astype(np.float32)

    # Host-side positives: pos[a][b][i] = x[a][i] . G_ab[i].
    pos = np.empty((2, 2, B), np.float64)
    for a in range(2):
        for b in range(2):
            G = _gather_positives(feats[b], lab[a], mlab[b])
            pos[a, b] = (x[a].astype(np.float64) *
                         G.astype(np.float64)).sum(axis=1)

    in_maps = []
    for c in range(NCORES):
        memB = np.empty([2, 2, 128, KT, 512], FP8_NP)
        for b in range(2):
            for nt in range(2):
                b_rows = feats[b][c * NS + nt * 512:c * NS + (nt + 1) * 512, :]
                memB[b, nt] = (b_rows.T * FP8_SCALE).reshape(
                    KT, 128, 512).transpose(1, 0, 2).astype(FP8_NP)
        in_maps.append({
            "xT": xT,
            "memB": memB,
            "nshift": nshift,
        })
    return in_maps, (shift, pos)


def combine(results, aux, targets_rgb, targets_ir,
            prototype_labels_rgb, prototype_labels_ir):
    shift, pos = aux
    rs = np.stack([np.asarray(r["res"], np.float64) for r in results])
    rs_sum = rs.sum(axis=0)                                    # [128, NOUT]
    sumexp = np.zeros((B, 4), np.float64)
    for si, (b, mt, a) in enumerate(SG_ORDER):
        c = a * 2 + b
        col = rs_sum[:, si] if si < 7 else rs_sum[:, 7] + rs_sum[:, 8]
        sumexp[mt * 128:(mt + 1) * 128, c] = col

    lab = [np.asarray(targets_rgb).astype(np.int64),
           np.asarray(targets_ir).astype(np.int64)]
    mlab = [np.asarray(prototype_labels_rgb).astype(np.int64),
            np.asarray(prototype_labels_ir).astype(np.int64)]

    losses = np.zeros(4, np.float64)
    for a in range(2):
        for b in range(2):
            c = a * 2 + b
            lse = shift[:, a, b] + np.log(sumexp[:, c])
            cnt = np.bincount(mlab[b], minlength=1 << 14)[
                np.clip(lab[a], 0, (1 << 14) - 1)].astype(np.float64)
            mlpp = (pos[a, b] / SUPCON_T - cnt * lse) / np.maximum(cnt, 1.0)
            losses[c] = -mlpp.mean()

    loss_contr = losses[0] + losses[3]        # (rgb,rgb) + (ir,ir)
    loss_cross = losses[1] + losses[2]        # (rgb,ir)  + (ir,rgb)
    return np.asarray([loss_contr, loss_cross], np.float32)


def run_device(in_maps, **kwargs):
    return run_bass_kernel_spmd(get_nc(), in_maps,
                                core_ids=list(range(NCORES)), **kwargs)


def kernel(inputs_rgb, inputs_ir, targets_rgb, targets_ir,
           features_rgb, features_ir,
           prototype_labels_rgb, prototype_labels_ir):
    in_maps, aux = make_in_maps(inputs_rgb, inputs_ir, targets_rgb,
                                targets_ir, features_rgb, features_ir,
                                prototype_labels_rgb, prototype_labels_ir)
    results = run_device(in_maps).results
    return combine(results, aux, targets_rgb, targets_ir,
                   prototype_labels_rgb, prototype_labels_ir)

